# revision 11
# baseline (speedup 1.0000x reference)
"""Trainium2 Bass kernel for a seq2seq decoder step (embed -> LSTM cell ->
Bahdanau attention -> projection -> 50k-vocab output GEMM).

Sharding (8 NeuronCores):
  - Batch-parallel (8 rows/core) for embedding, LSTM, attention, projection.
  - AllGather of cat_project [8,256] -> [64,256] on-device.
  - Vocab-parallel output GEMM: core c computes word_dist[:, c*6250:(c+1)*6250].

Host-side prep (inside kernel(), part of sharding): weight matrices are passed
pre-transposed so every GEMM contraction dim lands on SBUF partitions; the
encoder output slice is passed transposed [512, 3200] per core for the same
reason.  All compute (gather, LSTM, attention, softmax, GEMMs) runs on device.
"""

import numpy as np
from contextlib import ExitStack

import concourse.bass as bass
import concourse.bacc as bacc
import concourse.tile as tile
from concourse import mybir
from concourse.bass_utils import run_bass_kernel_spmd
from concourse.masks import make_identity

AF = mybir.ActivationFunctionType
ALU = mybir.AluOpType
F32 = mybir.dt.float32
I32 = mybir.dt.int32

NCORES = 8
B = 64          # full batch
BC = B // NCORES  # batch rows per core
S = 400         # source length
E = 128         # embedding dim
H = 256         # hidden dim
H2 = 2 * H      # encoder feature dim
H3 = 3 * H
H4 = 4 * H
V = 50000
VC = V // NCORES  # vocab rows per core
RC = BC * S       # encoder rows per core (3200)
VC_T = (VC + 127) // 128  # 49 vocab chunks of <=128
VC_LAST = VC - (VC_T - 1) * 128  # 106

# matmul input dtype (float32 for exact numerics; float32r is 4x faster on PE
# for moving free dim >= 256)
MM_DT = F32


def _mm(ap):
    """View an fp32 AP in the matmul input dtype."""
    if MM_DT is F32:
        return ap
    return ap.bitcast(MM_DT)


def _build_program():
    import os
    stage = int(os.environ.get("KSTAGE", "4"))  # 1=LSTM 2=+attn 3=+proj/ag 4=full
    nc = bacc.Bacc("TRN2", target_bir_lowering=False, debug=False,
                   num_devices=NCORES)

    # ---- I/O ----
    d_word = nc.dram_tensor("word", [BC, 1], I32, kind="ExternalInput")
    d_embed = nc.dram_tensor("embed", [V, E], F32, kind="ExternalInput")
    d_hT = nc.dram_tensor("hiddenT", [H, BC], F32, kind="ExternalInput")
    d_cT = nc.dram_tensor("cellT", [H, BC], F32, kind="ExternalInput")
    d_encT = nc.dram_tensor("encT", [H2, RC], F32, kind="ExternalInput")
    d_WihT = nc.dram_tensor("W_ihT", [E, H4], F32, kind="ExternalInput")
    d_WhhT = nc.dram_tensor("W_hhT", [H, H4], F32, kind="ExternalInput")
    d_biasg = nc.dram_tensor("bias_g", [128, 8], F32, kind="ExternalInput")
    d_WencT = nc.dram_tensor("W_encT", [H2, H], F32, kind="ExternalInput")
    d_WhT = nc.dram_tensor("W_hT", [H, H], F32, kind="ExternalInput")
    d_attnb = nc.dram_tensor("attn_b2", [128, 2], F32, kind="ExternalInput")
    d_v = nc.dram_tensor("v2", [128, 2], F32, kind="ExternalInput")
    d_projWT = nc.dram_tensor("proj_WT", [H3, H], F32, kind="ExternalInput")
    d_projb = nc.dram_tensor("proj_b1", [1, H], F32, kind="ExternalInput")
    d_outWT = nc.dram_tensor("out_WT", [H, VC], F32, kind="ExternalInput")
    d_outb = nc.dram_tensor("out_b49", [128, VC_T], F32, kind="ExternalInput")

    d_wdT = nc.dram_tensor("word_distT", [VC, B], F32, kind="ExternalOutput")
    d_ht_out = nc.dram_tensor("h_t", [BC, H], F32, kind="ExternalOutput")
    d_ct_out = nc.dram_tensor("c_t", [BC, H], F32, kind="ExternalOutput")

    with tile.TileContext(nc) as tc, ExitStack() as ctx:
        const = ctx.enter_context(tc.tile_pool(name="const", bufs=1))
        work = ctx.enter_context(tc.tile_pool(name="work", bufs=3))
        ps_pre = ctx.enter_context(tc.tile_pool(name="ps_pre", bufs=2, space="PSUM"))
        ps_misc = ctx.enter_context(tc.tile_pool(name="ps_misc", bufs=2, space="PSUM"))
        ps_tiny = ctx.enter_context(tc.tile_pool(name="ps_tiny", bufs=2, space="PSUM"))
        dram = ctx.enter_context(tc.tile_pool(name="dram", bufs=1, space="DRAM"))

        # ---- persistent SBUF tiles + loads ----
        identity = const.tile([128, 128], F32)
        make_identity(nc, identity[:])
        ones = const.tile([1, 128], F32)
        nc.vector.memset(ones[:], 1.0)

        widx = const.tile([BC, 1], I32)
        nc.sync.dma_start(out=widx[:], in_=d_word[:, :])

        hT_sb = const.tile([128, 2 * BC], F32)
        cT_sb = const.tile([128, 2 * BC], F32)
        for hc in range(2):
            nc.sync.dma_start(out=hT_sb[:, hc * BC:(hc + 1) * BC],
                              in_=d_hT[hc * 128:(hc + 1) * 128, :])
            nc.sync.dma_start(out=cT_sb[:, hc * BC:(hc + 1) * BC],
                              in_=d_cT[hc * 128:(hc + 1) * 128, :])

        WihT_sb = const.tile([128, H4], F32)
        nc.sync.dma_start(out=WihT_sb[:], in_=d_WihT[:, :])
        WhhT_sb = [const.tile([128, H4], F32, name=f"whh{i}") for i in range(2)]
        for hc in range(2):
            nc.sync.dma_start(out=WhhT_sb[hc][:],
                              in_=d_WhhT[hc * 128:(hc + 1) * 128, :])
        biasg_sb = const.tile([128, 8], F32)
        nc.sync.dma_start(out=biasg_sb[:], in_=d_biasg[:, :])

        WencT_sb = const.tile([128, 4 * H], F32)
        for fc in range(4):
            nc.sync.dma_start(out=WencT_sb[:, fc * H:(fc + 1) * H],
                              in_=d_WencT[fc * 128:(fc + 1) * 128, :])
        WhT_sb = const.tile([128, 2 * H], F32)
        for hc in range(2):
            nc.sync.dma_start(out=WhT_sb[:, hc * H:(hc + 1) * H],
                              in_=d_WhT[hc * 128:(hc + 1) * 128, :])
        attnb_sb = const.tile([128, 2], F32)
        nc.sync.dma_start(out=attnb_sb[:], in_=d_attnb[:, :])
        v_sb = const.tile([128, 2], F32)
        nc.sync.dma_start(out=v_sb[:], in_=d_v[:, :])
        projWT_sb = const.tile([128, 6 * H], F32)
        for cc in range(6):
            nc.sync.dma_start(out=projWT_sb[:, cc * H:(cc + 1) * H],
                              in_=d_projWT[cc * 128:(cc + 1) * 128, :])
        projb_sb = const.tile([1, H], F32)
        nc.sync.dma_start(out=projb_sb[:], in_=d_projb[:, :])

        encT_big = const.tile([128, 4 * RC], F32)  # f-chunk fc at cols fc*RC
        for fc in range(4):
            nc.sync.dma_start(out=encT_big[:, fc * RC:(fc + 1) * RC],
                              in_=d_encT[fc * 128:(fc + 1) * 128, :])

        outWT_sb = [const.tile([128, VC], F32, name=f"outWT{i}") for i in range(2)]
        for hc in range(2):
            nc.sync.dma_start(out=outWT_sb[hc][:],
                              in_=d_outWT[hc * 128:(hc + 1) * 128, :])
        outb_sb = const.tile([128, VC_T], F32)
        nc.sync.dma_start(out=outb_sb[:], in_=d_outb[:, :])

        # =====================  Phase A: embedding + LSTM  ====================
        embed_sb = const.tile([BC, E], F32)
        nc.gpsimd.indirect_dma_start(
            out=embed_sb[:], out_offset=None,
            in_=d_embed[:, :],
            in_offset=bass.IndirectOffsetOnAxis(ap=widx[:, :1], axis=0),
        )
        ps_x = ps_tiny.tile([128, 128], F32, name="ps_xT", tag="tiny")
        nc.tensor.transpose(out=ps_x[:, :BC], in_=embed_sb[:],
                            identity=identity[:BC, :BC])
        xT_sb = const.tile([128, BC], F32)
        nc.vector.tensor_copy(xT_sb[:], ps_x[:, :BC])

        # gatesT chunks: 0,1 -> i ; 2,3 -> f ; 4,5 -> g ; 6,7 -> o
        gate_sb = const.tile([128, 8 * BC], F32)  # col block gc*BC
        gate_fn = [AF.Sigmoid, AF.Sigmoid, AF.Sigmoid, AF.Sigmoid,
                   AF.Tanh, AF.Tanh, AF.Sigmoid, AF.Sigmoid]
        for gc in range(8):
            ps_g = ps_tiny.tile([128, 128], F32, name="ps_g", tag="tiny")
            nc.tensor.matmul(ps_g[:, :BC],
                             _mm(WihT_sb[:, gc * 128:(gc + 1) * 128]),
                             _mm(xT_sb[:]), start=True, stop=False)
            nc.tensor.matmul(ps_g[:, :BC],
                             _mm(WhhT_sb[0][:, gc * 128:(gc + 1) * 128]),
                             _mm(hT_sb[:, 0:BC]), start=False, stop=False)
            nc.tensor.matmul(ps_g[:, :BC],
                             _mm(WhhT_sb[1][:, gc * 128:(gc + 1) * 128]),
                             _mm(hT_sb[:, BC:2 * BC]), start=False, stop=True)
            nc.scalar.activation(gate_sb[:, gc * BC:(gc + 1) * BC], ps_g[:, :BC],
                                 gate_fn[gc], bias=biasg_sb[:, gc:gc + 1])

        i_sb = gate_sb[:, 0:2 * BC]
        f_sb = gate_sb[:, 2 * BC:4 * BC]
        g_sb = gate_sb[:, 4 * BC:6 * BC]
        o_sb = gate_sb[:, 6 * BC:8 * BC]

        cnewT = const.tile([128, 2 * BC], F32)
        tmp_fc = work.tile([128, 2 * BC], F32, name="tmp_fc")
        nc.vector.tensor_tensor(out=tmp_fc[:], in0=f_sb, in1=cT_sb[:], op=ALU.mult)
        tmp_ig = work.tile([128, 2 * BC], F32, name="tmp_ig")
        nc.vector.tensor_tensor(out=tmp_ig[:], in0=i_sb, in1=g_sb, op=ALU.mult)
        nc.vector.tensor_tensor(out=cnewT[:], in0=tmp_fc[:], in1=tmp_ig[:], op=ALU.add)

        tanh_c = work.tile([128, 2 * BC], F32, name="tanh_c")
        nc.scalar.activation(tanh_c[:], cnewT[:], AF.Tanh)
        hraw = work.tile([128, 2 * BC], F32, name="hraw")
        nc.vector.tensor_tensor(out=hraw[:], in0=o_sb, in1=tanh_c[:], op=ALU.mult)
        htT = const.tile([128, 2 * BC], F32)
        nc.scalar.activation(htT[:], hraw[:], AF.Tanh)  # extra tanh from the model

        # h_t / c_t outputs (un-transpose)
        ht_out_sb = const.tile([BC, H], F32)
        ct_out_sb = const.tile([BC, H], F32)
        for hc in range(2):
            ps_t = ps_tiny.tile([128, 128], F32, name="ps_unt", tag="tiny")
            nc.tensor.transpose(out=ps_t[:BC, :128],
                                in_=htT[:, hc * BC:(hc + 1) * BC],
                                identity=identity[:, :])
            nc.vector.tensor_copy(ht_out_sb[:, hc * 128:(hc + 1) * 128],
                                  ps_t[:BC, :128])
            ps_t2 = ps_tiny.tile([128, 128], F32, name="ps_unt", tag="tiny")
            nc.tensor.transpose(out=ps_t2[:BC, :128],
                                in_=cnewT[:, hc * BC:(hc + 1) * BC],
                                identity=identity[:, :])
            nc.vector.tensor_copy(ct_out_sb[:, hc * 128:(hc + 1) * 128],
                                  ps_t2[:BC, :128])
        nc.sync.dma_start(out=d_ht_out[:, :], in_=ht_out_sb[:])
        nc.sync.dma_start(out=d_ct_out[:, :], in_=ct_out_sb[:])

        # qT = W_h @ h_tT + attn_b   [256, BC]
        qT_sb = const.tile([128, 2 * BC], F32)
        for hc2 in range(2):
            ps_q = ps_tiny.tile([128, 128], F32, name="ps_q", tag="tiny")
            for hcp in range(2):
                nc.tensor.matmul(ps_q[:, :BC],
                                 _mm(WhT_sb[:, hcp * H + hc2 * 128:
                                            hcp * H + (hc2 + 1) * 128]),
                                 _mm(htT[:, hcp * BC:(hcp + 1) * BC]),
                                 start=(hcp == 0), stop=(hcp == 1))
            nc.scalar.activation(qT_sb[:, hc2 * BC:(hc2 + 1) * BC], ps_q[:, :BC],
                                 AF.Identity, bias=attnb_sb[:, hc2:hc2 + 1])

        # catT: chunks 0-3 = contextT (f), 4-5 = h_tT
        catT_sb = const.tile([128, 6 * BC], F32)
        nc.vector.tensor_copy(catT_sb[:, 4 * BC:6 * BC], htT[:])

        # =====================  Phase B: attention per batch  =================
        for b in (range(BC) if stage >= 2 else []):
            tanhT = [work.tile([128, S], F32, name=f"tanhT{hc}") for hc in range(2)]
            for hc in range(2):
                ps_p = ps_pre.tile([128, S], F32, name="ps_pre")
                for fc in range(4):
                    nc.tensor.matmul(
                        ps_p[:],
                        _mm(WencT_sb[:, fc * H + hc * 128:fc * H + (hc + 1) * 128]),
                        _mm(encT_big[:, fc * RC + b * S:fc * RC + (b + 1) * S]),
                        start=(fc == 0), stop=(fc == 3))
                nc.scalar.activation(tanhT[hc][:], ps_p[:], AF.Tanh,
                                     bias=qT_sb[:, hc * BC + b:hc * BC + b + 1])

            ps_e = ps_misc.tile([128, S], F32, name="ps_e", tag="misc")
            nc.tensor.matmul(ps_e[:1, :], _mm(v_sb[:, 0:1]), _mm(tanhT[0][:]),
                             start=True, stop=False)
            nc.tensor.matmul(ps_e[:1, :], _mm(v_sb[:, 1:2]), _mm(tanhT[1][:]),
                             start=False, stop=True)

            negmax = work.tile([1, 1], F32, name="negmax")
            nc.vector.tensor_reduce(out=negmax[:], in_=ps_e[:1, :],
                                    axis=mybir.AxisListType.X, op=ALU.max,
                                    negate=True)
            exp_sb = work.tile([1, S], F32, name="exp_sb")
            sumexp = work.tile([1, 1], F32, name="sumexp")
            nc.scalar.activation(exp_sb[:], ps_e[:1, :], AF.Exp,
                                 bias=negmax[:], accum_out=sumexp[:])
            rsum = work.tile([1, 1], F32, name="rsum")
            nc.vector.reciprocal(rsum[:], sumexp[:])
            attn_sb = work.tile([1, S], F32, name="attn_sb")
            nc.vector.tensor_scalar_mul(attn_sb[:], exp_sb[:], rsum[:])

            ps_bc = ps_misc.tile([128, S], F32, name="ps_bc", tag="misc")
            nc.tensor.matmul(ps_bc[:], _mm(ones[:1, :]), _mm(attn_sb[:]),
                             start=True, stop=True)

            bc_sb = work.tile([128, S], F32, name="bc_sb")
            nc.vector.tensor_copy(bc_sb[:], ps_bc[:])
            scr = work.tile([128, 4 * S], F32, name="ttr_scr")
            nc.vector.tensor_tensor(
                out=scr[:].rearrange("p (c s) -> p c s", s=S),
                in0=encT_big[:].rearrange("p (c r) -> p c r", r=RC)
                    [:, :, b * S:(b + 1) * S],
                in1=bc_sb[:].unsqueeze(1).broadcast_to([128, 4, S]),
                op=ALU.mult)
            ctx4 = work.tile([128, 4], F32, name="ctx4")
            nc.vector.tensor_reduce(
                out=ctx4[:], in_=scr[:].rearrange("p (c s) -> p c s", s=S),
                axis=mybir.AxisListType.X, op=ALU.add)
            nc.vector.tensor_copy(
                catT_sb[:].rearrange("p (c j) -> p c j", j=BC)[:, 0:4, b:b + 1],
                ctx4[:].unsqueeze(2))

        # =====================  Phase C: projection + AllGather  ==============
        do_cd = stage >= 3
        ps_cp = ps_misc.tile([128, S], F32, name="ps_cp", tag="misc")
        cpT_sb = const.tile([128, 2 * B], F32)  # col block hc*B
        if do_cd:
            nc.tensor.matmul(ps_cp[:BC, :H], _mm(ones[:1, :BC]), _mm(projb_sb[:]),
                             start=True, stop=False)
            for cc in range(6):
                nc.tensor.matmul(ps_cp[:BC, :H],
                                 _mm(catT_sb[:, cc * BC:(cc + 1) * BC]),
                                 _mm(projWT_sb[:, cc * H:(cc + 1) * H]),
                                 start=False, stop=(cc == 5))
            cp_sb = const.tile([BC, H], F32)
            nc.vector.tensor_copy(cp_sb[:], ps_cp[:BC, :H])

            cp_bounce = dram.tile([BC, H], F32)
            cpall_bounce = dram.tile([B, H], F32)
            nc.gpsimd.dma_start(out=cp_bounce[:], in_=cp_sb[:])
            nc.gpsimd.collective_compute(
                "AllGather", ALU.bypass,
                replica_groups=[list(range(NCORES))],
                ins=[cp_bounce.opt()],
                outs=[cpall_bounce.opt()],
            )
            cpall_sb = const.tile([B, H], F32)
            nc.gpsimd.dma_start(out=cpall_sb[:], in_=cpall_bounce[:])

            for hc in range(2):
                ps_t3 = ps_tiny.tile([128, 128], F32, name="ps_cpT", tag="tiny")
                nc.tensor.transpose(out=ps_t3[:, :B],
                                    in_=cpall_sb[:, hc * 128:(hc + 1) * 128],
                                    identity=identity[:B, :B])
                nc.vector.tensor_copy(cpT_sb[:, hc * B:(hc + 1) * B], ps_t3[:, :B])
        else:
            nc.vector.memset(cpT_sb[:], 0.0)

        # =====================  Phase D: output GEMM  =========================
        wd_sb = const.tile([128, VC_T * B], F32)
        for wc in (range(VC_T) if stage >= 4 else []):
            m = 128 if wc < VC_T - 1 else VC_LAST
            ps_w = ps_tiny.tile([128, 128], F32, name="ps_wd", tag="tiny")
            for hc in range(2):
                nc.tensor.matmul(ps_w[:m, :B],
                                 _mm(outWT_sb[hc][:, wc * 128:wc * 128 + m]),
                                 _mm(cpT_sb[:, hc * B:(hc + 1) * B]),
                                 start=(hc == 0), stop=(hc == 1))
            nc.vector.tensor_scalar_add(wd_sb[:m, wc * B:(wc + 1) * B],
                                        ps_w[:m, :B], outb_sb[:m, wc:wc + 1])

        if stage < 4:
            nc.vector.memset(wd_sb[:], 0.0)
        n_full = VC_T - 1  # 48 full 128-row chunks
        nc.sync.dma_start(
            out=d_wdT[0:n_full * 128, :].rearrange("(w p) j -> p w j", p=128),
            in_=wd_sb[:, 0:n_full * B].rearrange("p (w j) -> p w j", j=B))
        nc.sync.dma_start(
            out=d_wdT[n_full * 128:VC, :],
            in_=wd_sb[:VC_LAST, n_full * B:VC_T * B])

    nc.compile()
    return nc


_cached_nc = None


def _ensure_ntff_hook():
    """This image's `antenv` lacks `axon_hooks`, so trace=True crashes in
    bass_utils.  Recreate the module and register the ctypes NTFF hook the
    boot code would have installed."""
    import sys
    import types
    try:
        from antenv.axon_hooks import get_axon_ntff_profile_hook  # noqa: F401
        return
    except ImportError:
        pass
    import antenv
    mod = types.ModuleType("antenv.axon_hooks")
    mod._hook = None
    def set_axon_ntff_profile_hook(h):
        mod._hook = h
    def get_axon_ntff_profile_hook():
        return mod._hook
    mod.set_axon_ntff_profile_hook = set_axon_ntff_profile_hook
    mod.get_axon_ntff_profile_hook = get_axon_ntff_profile_hook
    sys.modules["antenv.axon_hooks"] = mod
    antenv.axon_hooks = mod
    try:
        from trn_agent_boot.trn_boot import _ntff_profile_via_ctypes
        hook = _ntff_profile_via_ctypes("/opt/axon/libaxon_pjrt.so")
        if hook is not None:
            mod._hook = hook
    except Exception:
        pass


def _get_nc():
    global _cached_nc
    if _cached_nc is None:
        _cached_nc = _build_program()
    return _cached_nc


last_results = None  # BassKernelResults of the most recent run (for profiling)


def kernel(word, hidden, cell, encoder_output, max_source_len,
           embed_table, W_ih, W_hh, b_ih, b_hh,
           attn_W, attn_b, v, proj_W, proj_b, out_W, out_b,
           trace=False):
    global last_results
    word = np.asarray(word).astype(np.int32)
    hidden = np.asarray(hidden, dtype=np.float32)
    cell = np.asarray(cell, dtype=np.float32)
    enc = np.asarray(encoder_output, dtype=np.float32)
    embed_table = np.ascontiguousarray(np.asarray(embed_table, dtype=np.float32))
    W_ihT = np.ascontiguousarray(np.asarray(W_ih, dtype=np.float32).T)
    W_hhT = np.ascontiguousarray(np.asarray(W_hh, dtype=np.float32).T)
    bias_g = np.ascontiguousarray(
        (np.asarray(b_ih, dtype=np.float32)
         + np.asarray(b_hh, dtype=np.float32)).reshape(8, 128).T)
    attn_W = np.asarray(attn_W, dtype=np.float32)
    W_encT = np.ascontiguousarray(attn_W[:, :H2].T)
    W_hT = np.ascontiguousarray(attn_W[:, H2:].T)
    attn_b2 = np.ascontiguousarray(
        np.asarray(attn_b, dtype=np.float32).reshape(2, 128).T)
    v2 = np.ascontiguousarray(np.asarray(v, dtype=np.float32).reshape(2, 128).T)
    proj_WT = np.ascontiguousarray(np.asarray(proj_W, dtype=np.float32).T)
    proj_b1 = np.ascontiguousarray(
        np.asarray(proj_b, dtype=np.float32).reshape(1, H))
    out_W = np.asarray(out_W, dtype=np.float32)
    out_b = np.asarray(out_b, dtype=np.float32)

    in_maps = []
    for c in range(NCORES):
        bs = slice(c * BC, (c + 1) * BC)
        vs = slice(c * VC, (c + 1) * VC)
        outb_pad = np.zeros(VC_T * 128, dtype=np.float32)
        outb_pad[:VC] = out_b[vs]
        in_maps.append({
            "word": np.ascontiguousarray(word[bs]),
            "embed": embed_table,
            "hiddenT": np.ascontiguousarray(hidden[bs].T),
            "cellT": np.ascontiguousarray(cell[bs].T),
            "encT": np.ascontiguousarray(enc[bs].reshape(RC, H2).T),
            "W_ihT": W_ihT,
            "W_hhT": W_hhT,
            "bias_g": bias_g,
            "W_encT": W_encT,
            "W_hT": W_hT,
            "attn_b2": attn_b2,
            "v2": v2,
            "proj_WT": proj_WT,
            "proj_b1": proj_b1,
            "out_WT": np.ascontiguousarray(out_W[vs].T),
            "out_b49": np.ascontiguousarray(outb_pad.reshape(VC_T, 128).T),
        })

    nc = _get_nc()
    if trace:
        _ensure_ntff_hook()
    res = run_bass_kernel_spmd(nc, in_maps, core_ids=list(range(NCORES)),
                               trace=trace)
    last_results = res

    word_dist = np.empty((B, V), dtype=np.float32)
    h_t = np.empty((B, H), dtype=np.float32)
    c_t = np.empty((B, H), dtype=np.float32)
    for c in range(NCORES):
        out = res.results[c]
        word_dist[:, c * VC:(c + 1) * VC] = out["word_distT"].T
        h_t[c * BC:(c + 1) * BC] = out["h_t"]
        c_t[c * BC:(c + 1) * BC] = out["c_t"]
    return (word_dist, h_t, c_t)


# revision 15
# speedup vs baseline: 1.2588x; 1.2588x over previous
"""Trainium2 Bass kernel for a seq2seq decoder step (embed -> LSTM cell ->
Bahdanau attention -> projection -> 50k-vocab output GEMM).

Sharding (8 NeuronCores):
  - Batch-parallel (8 rows/core) for embedding, LSTM, attention, projection.
  - AllGather of cat_project [8,256] -> [64,256] on-device.
  - Vocab-parallel output GEMM: core c computes word_dist[:, c*6250:(c+1)*6250].

Host-side prep (inside kernel(), part of sharding): weight matrices are passed
pre-transposed so every GEMM contraction dim lands on SBUF partitions; the
encoder output slice is passed transposed [512, 3200] per core for the same
reason.  All compute (gather, LSTM, attention, softmax, GEMMs) runs on device.
"""

import numpy as np
from contextlib import ExitStack

import concourse.bass as bass
import concourse.bacc as bacc
import concourse.tile as tile
from concourse import mybir
from concourse.bass_utils import run_bass_kernel_spmd
from concourse.masks import make_identity

AF = mybir.ActivationFunctionType
ALU = mybir.AluOpType
F32 = mybir.dt.float32
I32 = mybir.dt.int32

NCORES = 8
B = 64          # full batch
BC = B // NCORES  # batch rows per core
S = 400         # source length
E = 128         # embedding dim
H = 256         # hidden dim
H2 = 2 * H      # encoder feature dim
H3 = 3 * H
H4 = 4 * H
V = 50000
VC = V // NCORES  # vocab rows per core
RC = BC * S       # encoder rows per core (3200)
VC_T = (VC + 127) // 128  # 49 vocab chunks of <=128
VC_LAST = VC - (VC_T - 1) * 128  # 106

# matmul input dtype: f32 (exact, 4 cyc/row), f32r (1 cyc/row at N>=256,
# reduced precision), bf16 (1 cyc/row, FWL fast weight load, lowest precision).
# All tiles feeding matmuls are typed T32 and their producers round into it.
import os as _os
_KMMDT = _os.environ.get("KMMDT", "f32r")
T32 = {"f32": F32, "f32r": mybir.dt.float32r,
       "bf16": mybir.dt.bfloat16}[_KMMDT]


def _mm(ap):
    return ap


def _build_program():
    import os
    cast_load = T32 is not F32
    stage = int(os.environ.get("KSTAGE", "4"))  # 1=LSTM 2=+attn 3=+proj/ag 4=full
    nc = bacc.Bacc("TRN2", target_bir_lowering=False, debug=False,
                   num_devices=NCORES)

    # ---- I/O ----
    d_word = nc.dram_tensor("word", [BC, 1], I32, kind="ExternalInput")
    d_embed = nc.dram_tensor("embed", [V, E], F32, kind="ExternalInput")
    d_hT = nc.dram_tensor("hiddenT", [H, BC], F32, kind="ExternalInput")
    d_cT = nc.dram_tensor("cellT", [H, BC], F32, kind="ExternalInput")
    d_encT = nc.dram_tensor("encT", [H2, RC], F32, kind="ExternalInput")
    d_WihT = nc.dram_tensor("W_ihT", [E, H4], F32, kind="ExternalInput")
    d_WhhT = nc.dram_tensor("W_hhT", [H, H4], F32, kind="ExternalInput")
    d_biasg = nc.dram_tensor("bias_g", [128, 8], F32, kind="ExternalInput")
    d_WencT = nc.dram_tensor("W_encT", [H2, H], F32, kind="ExternalInput")
    d_WhT = nc.dram_tensor("W_hT", [H, H], F32, kind="ExternalInput")
    d_attnb = nc.dram_tensor("attn_b2", [128, 2], F32, kind="ExternalInput")
    d_v = nc.dram_tensor("v2", [128, 2], F32, kind="ExternalInput")
    d_projWT = nc.dram_tensor("proj_WT", [H3, H], F32, kind="ExternalInput")
    d_projb = nc.dram_tensor("proj_b1", [1, H], F32, kind="ExternalInput")
    d_outWT = nc.dram_tensor("out_WT", [H, VC], F32, kind="ExternalInput")
    d_outb = nc.dram_tensor("out_b1v", [1, VC], F32, kind="ExternalInput")

    d_wd = nc.dram_tensor("word_dist", [B, VC], F32, kind="ExternalOutput")
    d_ht_out = nc.dram_tensor("h_t", [BC, H], F32, kind="ExternalOutput")
    d_ct_out = nc.dram_tensor("c_t", [BC, H], F32, kind="ExternalOutput")

    with tile.TileContext(nc) as tc, ExitStack() as ctx:
        const = ctx.enter_context(tc.tile_pool(name="const", bufs=1))
        work = ctx.enter_context(tc.tile_pool(name="work", bufs=2))
        ps_pre = ctx.enter_context(tc.tile_pool(name="ps_pre", bufs=2, space="PSUM"))
        ps_misc = ctx.enter_context(tc.tile_pool(name="ps_misc", bufs=2, space="PSUM"))
        ps_tiny = ctx.enter_context(tc.tile_pool(name="ps_tiny", bufs=2, space="PSUM"))
        dram = ctx.enter_context(tc.tile_pool(name="dram", bufs=1, space="DRAM"))

        def dma_ld(out, in_):
            if cast_load:
                nc.gpsimd.dma_start(out=out, in_=in_)
            else:
                nc.sync.dma_start(out=out, in_=in_)

        # ---- persistent SBUF tiles + loads ----
        identity = const.tile([128, 128], F32)
        make_identity(nc, identity[:])
        ones_f = const.tile([1, 128], F32)
        nc.vector.memset(ones_f[:], 1.0)
        ones = const.tile([1, 128], T32)
        nc.vector.tensor_copy(ones[:], ones_f[:])

        widx = const.tile([BC, 1], I32)
        nc.sync.dma_start(out=widx[:], in_=d_word[:, :])

        hT_sb = const.tile([128, 2 * BC], T32)
        cT_sb = const.tile([128, 2 * BC], F32)
        for hc in range(2):
            dma_ld(out=hT_sb[:, hc * BC:(hc + 1) * BC],
                              in_=d_hT[hc * 128:(hc + 1) * 128, :])
            nc.sync.dma_start(out=cT_sb[:, hc * BC:(hc + 1) * BC],
                              in_=d_cT[hc * 128:(hc + 1) * 128, :])

        WihT_sb = const.tile([128, H4], T32)
        dma_ld(out=WihT_sb[:], in_=d_WihT[:, :])
        WhhT_sb = [const.tile([128, H4], T32, name=f"whh{i}") for i in range(2)]
        for hc in range(2):
            dma_ld(out=WhhT_sb[hc][:],
                              in_=d_WhhT[hc * 128:(hc + 1) * 128, :])
        biasg_sb = const.tile([128, 8], F32)
        nc.sync.dma_start(out=biasg_sb[:], in_=d_biasg[:, :])

        WencT_sb = const.tile([128, 4 * H], T32)
        for fc in range(4):
            dma_ld(out=WencT_sb[:, fc * H:(fc + 1) * H],
                              in_=d_WencT[fc * 128:(fc + 1) * 128, :])
        WhT_sb = const.tile([128, 2 * H], T32)
        for hc in range(2):
            dma_ld(out=WhT_sb[:, hc * H:(hc + 1) * H],
                              in_=d_WhT[hc * 128:(hc + 1) * 128, :])
        attnb_sb = const.tile([128, 2], F32)
        nc.sync.dma_start(out=attnb_sb[:], in_=d_attnb[:, :])
        v_sb = const.tile([128, 2], T32)
        dma_ld(out=v_sb[:], in_=d_v[:, :])
        projWT_sb = const.tile([128, 6 * H], T32)
        for cc in range(6):
            dma_ld(out=projWT_sb[:, cc * H:(cc + 1) * H],
                              in_=d_projWT[cc * 128:(cc + 1) * 128, :])
        projb_sb = const.tile([1, H], T32)
        dma_ld(out=projb_sb[:], in_=d_projb[:, :])

        encT_big = const.tile([128, 4 * RC], T32)  # f-chunk fc at cols fc*RC
        for fc in range(4):
            dma_ld(out=encT_big[:, fc * RC:(fc + 1) * RC],
                              in_=d_encT[fc * 128:(fc + 1) * 128, :])

        outWT_sb = [const.tile([128, VC], T32, name=f"outWT{i}") for i in range(2)]
        for hc in range(2):
            dma_ld(out=outWT_sb[hc][:],
                              in_=d_outWT[hc * 128:(hc + 1) * 128, :])
        outb_sb = const.tile([1, VC], T32)
        dma_ld(out=outb_sb[:], in_=d_outb[:, :])

        # =====================  Phase A: embedding + LSTM  ====================
        embed_sb = const.tile([BC, E], F32)
        nc.gpsimd.indirect_dma_start(
            out=embed_sb[:], out_offset=None,
            in_=d_embed[:, :],
            in_offset=bass.IndirectOffsetOnAxis(ap=widx[:, :1], axis=0),
        )
        ps_x = ps_tiny.tile([128, 128], F32, name="ps_xT", tag="tiny")
        nc.tensor.transpose(out=ps_x[:, :BC], in_=embed_sb[:],
                            identity=identity[:BC, :BC])
        xT_sb = const.tile([128, BC], T32)
        nc.vector.tensor_copy(xT_sb[:], ps_x[:, :BC])

        # gatesT chunks: 0,1 -> i ; 2,3 -> f ; 4,5 -> g ; 6,7 -> o
        gate_sb = const.tile([128, 8 * BC], F32)  # col block gc*BC
        gate_fn = [AF.Sigmoid, AF.Sigmoid, AF.Sigmoid, AF.Sigmoid,
                   AF.Tanh, AF.Tanh, AF.Sigmoid, AF.Sigmoid]
        for gc in range(8):
            ps_g = ps_tiny.tile([128, 128], F32, name="ps_g", tag="tiny")
            nc.tensor.matmul(ps_g[:, :BC],
                             _mm(WihT_sb[:, gc * 128:(gc + 1) * 128]),
                             _mm(xT_sb[:]), start=True, stop=False)
            nc.tensor.matmul(ps_g[:, :BC],
                             _mm(WhhT_sb[0][:, gc * 128:(gc + 1) * 128]),
                             _mm(hT_sb[:, 0:BC]), start=False, stop=False)
            nc.tensor.matmul(ps_g[:, :BC],
                             _mm(WhhT_sb[1][:, gc * 128:(gc + 1) * 128]),
                             _mm(hT_sb[:, BC:2 * BC]), start=False, stop=True)
            nc.scalar.activation(gate_sb[:, gc * BC:(gc + 1) * BC], ps_g[:, :BC],
                                 gate_fn[gc], bias=biasg_sb[:, gc:gc + 1])

        i_sb = gate_sb[:, 0:2 * BC]
        f_sb = gate_sb[:, 2 * BC:4 * BC]
        g_sb = gate_sb[:, 4 * BC:6 * BC]
        o_sb = gate_sb[:, 6 * BC:8 * BC]

        cnewT = const.tile([128, 2 * BC], F32)
        tmp_fc = work.tile([128, 2 * BC], F32, name="tmp_fc")
        nc.vector.tensor_tensor(out=tmp_fc[:], in0=f_sb, in1=cT_sb[:], op=ALU.mult)
        tmp_ig = work.tile([128, 2 * BC], F32, name="tmp_ig")
        nc.vector.tensor_tensor(out=tmp_ig[:], in0=i_sb, in1=g_sb, op=ALU.mult)
        nc.vector.tensor_tensor(out=cnewT[:], in0=tmp_fc[:], in1=tmp_ig[:], op=ALU.add)

        tanh_c = work.tile([128, 2 * BC], F32, name="tanh_c")
        nc.scalar.activation(tanh_c[:], cnewT[:], AF.Tanh)
        hraw = work.tile([128, 2 * BC], F32, name="hraw")
        nc.vector.tensor_tensor(out=hraw[:], in0=o_sb, in1=tanh_c[:], op=ALU.mult)
        htT = const.tile([128, 2 * BC], T32)
        nc.scalar.activation(htT[:], hraw[:], AF.Tanh)  # extra tanh from the model

        # h_t / c_t outputs (un-transpose)
        ht_out_sb = const.tile([BC, H], F32)
        ct_out_sb = const.tile([BC, H], F32)
        for hc in range(2):
            ps_t = ps_tiny.tile([128, 128], F32, name="ps_unt", tag="tiny")
            nc.tensor.transpose(out=ps_t[:BC, :128],
                                in_=htT[:, hc * BC:(hc + 1) * BC].bitcast(F32)
                                if T32 is mybir.dt.float32r
                                else htT[:, hc * BC:(hc + 1) * BC],
                                identity=identity[:, :])
            nc.vector.tensor_copy(ht_out_sb[:, hc * 128:(hc + 1) * 128],
                                  ps_t[:BC, :128])
            ps_t2 = ps_tiny.tile([128, 128], F32, name="ps_unt", tag="tiny")
            nc.tensor.transpose(out=ps_t2[:BC, :128],
                                in_=cnewT[:, hc * BC:(hc + 1) * BC],
                                identity=identity[:, :])
            nc.vector.tensor_copy(ct_out_sb[:, hc * 128:(hc + 1) * 128],
                                  ps_t2[:BC, :128])
        nc.sync.dma_start(out=d_ht_out[:, :], in_=ht_out_sb[:])
        nc.sync.dma_start(out=d_ct_out[:, :], in_=ct_out_sb[:])

        # qT = W_h @ h_tT + attn_b   [256, BC]
        qT_sb = const.tile([128, 2 * BC], F32)
        for hc2 in range(2):
            ps_q = ps_tiny.tile([128, 128], F32, name="ps_q", tag="tiny")
            for hcp in range(2):
                nc.tensor.matmul(ps_q[:, :BC],
                                 _mm(WhT_sb[:, hcp * H + hc2 * 128:
                                            hcp * H + (hc2 + 1) * 128]),
                                 _mm(htT[:, hcp * BC:(hcp + 1) * BC]),
                                 start=(hcp == 0), stop=(hcp == 1))
            nc.scalar.activation(qT_sb[:, hc2 * BC:(hc2 + 1) * BC], ps_q[:, :BC],
                                 AF.Identity, bias=attnb_sb[:, hc2:hc2 + 1])

        # catT: chunks 0-3 = contextT (f), 4-5 = h_tT
        catT_sb = const.tile([128, 6 * BC], T32)
        nc.vector.tensor_copy(catT_sb[:, 4 * BC:6 * BC], htT[:])

        # =====================  Phase B: attention per batch  =================
        for b in (range(BC) if stage >= 2 else []):
            tanhT = [work.tile([128, S], T32, name=f"tanhT{hc}") for hc in range(2)]
            for hc in range(2):
                ps_p = ps_pre.tile([128, S], F32, name="ps_pre")
                for fc in range(4):
                    nc.tensor.matmul(
                        ps_p[:],
                        _mm(WencT_sb[:, fc * H + hc * 128:fc * H + (hc + 1) * 128]),
                        _mm(encT_big[:, fc * RC + b * S:fc * RC + (b + 1) * S]),
                        start=(fc == 0), stop=(fc == 3))
                nc.scalar.activation(tanhT[hc][:], ps_p[:], AF.Tanh,
                                     bias=qT_sb[:, hc * BC + b:hc * BC + b + 1])

            ps_e = ps_misc.tile([128, S], F32, name="ps_e", tag="misc")
            nc.tensor.matmul(ps_e[:1, :], _mm(v_sb[:, 0:1]), _mm(tanhT[0][:]),
                             start=True, stop=False)
            nc.tensor.matmul(ps_e[:1, :], _mm(v_sb[:, 1:2]), _mm(tanhT[1][:]),
                             start=False, stop=True)

            negmax = work.tile([1, 1], F32, name="negmax")
            nc.vector.tensor_reduce(out=negmax[:], in_=ps_e[:1, :],
                                    axis=mybir.AxisListType.X, op=ALU.max,
                                    negate=True)
            exp_sb = work.tile([1, S], F32, name="exp_sb")
            sumexp = work.tile([1, 1], F32, name="sumexp")
            nc.scalar.activation(exp_sb[:], ps_e[:1, :], AF.Exp,
                                 bias=negmax[:], accum_out=sumexp[:])
            rsum = work.tile([1, 1], F32, name="rsum")
            nc.vector.reciprocal(rsum[:], sumexp[:])
            attn_sb = work.tile([1, S], T32, name="attn_sb")
            nc.vector.tensor_scalar_mul(attn_sb[:], exp_sb[:], rsum[:])

            ps_bc = ps_misc.tile([128, S], F32, name="ps_bc", tag="misc")
            nc.tensor.matmul(ps_bc[:], _mm(ones[:1, :]), _mm(attn_sb[:]),
                             start=True, stop=True)

            bc_sb = work.tile([128, S], F32, name="bc_sb")
            nc.vector.tensor_copy(bc_sb[:], ps_bc[:])
            scr = work.tile([128, 4 * S], F32, name="ttr_scr")
            nc.vector.tensor_tensor(
                out=scr[:].rearrange("p (c s) -> p c s", s=S),
                in0=encT_big[:].rearrange("p (c r) -> p c r", r=RC)
                    [:, :, b * S:(b + 1) * S],
                in1=bc_sb[:].unsqueeze(1).broadcast_to([128, 4, S]),
                op=ALU.mult)
            ctx4 = work.tile([128, 4], F32, name="ctx4")
            nc.vector.tensor_reduce(
                out=ctx4[:], in_=scr[:].rearrange("p (c s) -> p c s", s=S),
                axis=mybir.AxisListType.X, op=ALU.add)
            nc.vector.tensor_copy(
                catT_sb[:].rearrange("p (c j) -> p c j", j=BC)[:, 0:4, b:b + 1],
                ctx4[:].unsqueeze(2))

        # =====================  Phase C: projection + AllGather  ==============
        do_cd = stage >= 3
        ps_cp = ps_misc.tile([128, S], F32, name="ps_cp", tag="misc")
        cpT_sb = const.tile([128, 2 * B], T32)  # col block hc*B
        if do_cd:
            nc.tensor.matmul(ps_cp[:BC, :H], _mm(ones[:1, :BC]), _mm(projb_sb[:]),
                             start=True, stop=False)
            for cc in range(6):
                nc.tensor.matmul(ps_cp[:BC, :H],
                                 _mm(catT_sb[:, cc * BC:(cc + 1) * BC]),
                                 _mm(projWT_sb[:, cc * H:(cc + 1) * H]),
                                 start=False, stop=(cc == 5))
            cp_sb = const.tile([BC, H], F32)
            nc.vector.tensor_copy(cp_sb[:], ps_cp[:BC, :H])

            cp_bounce = dram.tile([BC, H], F32)
            cpall_bounce = dram.tile([B, H], F32)
            nc.gpsimd.dma_start(out=cp_bounce[:], in_=cp_sb[:])
            nc.gpsimd.collective_compute(
                "AllGather", ALU.bypass,
                replica_groups=[list(range(NCORES))],
                ins=[cp_bounce.opt()],
                outs=[cpall_bounce.opt()],
            )
            cpall_sb = const.tile([B, H], F32)
            nc.gpsimd.dma_start(out=cpall_sb[:], in_=cpall_bounce[:])

            for hc in range(2):
                ps_t3 = ps_tiny.tile([128, 128], F32, name="ps_cpT", tag="tiny")
                nc.tensor.transpose(out=ps_t3[:, :B],
                                    in_=cpall_sb[:, hc * 128:(hc + 1) * 128],
                                    identity=identity[:B, :B])
                nc.vector.tensor_copy(cpT_sb[:, hc * B:(hc + 1) * B], ps_t3[:, :B])
        else:
            nc.vector.tensor_copy(cpT_sb[:], ones_f[:].to_broadcast([128, 2 * B]))

        # =====================  Phase D: output GEMM  =========================
        # word_dist[b, w] = cp[b, :] @ out_WT[:, w] + out_b[w]
        # psum [64, 512]: bias via ones-matmul, then 2 h-chunk accumulations
        WCHUNK = 512
        NWC = (VC + WCHUNK - 1) // WCHUNK  # 13 (12x512 + 106)
        for wc in (range(NWC) if stage >= 4 else []):
            w0 = wc * WCHUNK
            n = min(WCHUNK, VC - w0)
            ps_w = ps_misc.tile([B, WCHUNK], F32, name="ps_wd", tag="wd")
            nc.tensor.matmul(ps_w[:, :n], _mm(ones[:1, :B]),
                             _mm(outb_sb[:1, w0:w0 + n]),
                             start=True, stop=False)
            for hc in range(2):
                nc.tensor.matmul(ps_w[:, :n],
                                 _mm(cpT_sb[:, hc * B:(hc + 1) * B]),
                                 _mm(outWT_sb[hc][:, w0:w0 + n]),
                                 start=False, stop=(hc == 1))
            wd_c = work.tile([B, WCHUNK], F32, name="wd_c", tag="wd_sb", bufs=3)
            nc.vector.tensor_copy(wd_c[:, :n], ps_w[:, :n])
            nc.sync.dma_start(out=d_wd[:, w0:w0 + n], in_=wd_c[:, :n])

        if stage < 4:
            zz = work.tile([B, VC // 5], F32, name="zz")
            for piece in range(5):
                nc.vector.memset(zz[:], 0.0)
                nc.sync.dma_start(out=d_wd[:, piece * (VC // 5):(piece + 1) * (VC // 5)],
                                  in_=zz[:])

    nc.compile()
    return nc


_cached_nc = None


def _ensure_ntff_hook():
    """This image's `antenv` lacks `axon_hooks`, so trace=True crashes in
    bass_utils.  Recreate the module and register the ctypes NTFF hook the
    boot code would have installed."""
    import sys
    import types
    try:
        from antenv.axon_hooks import get_axon_ntff_profile_hook  # noqa: F401
        return
    except ImportError:
        pass
    import antenv
    mod = types.ModuleType("antenv.axon_hooks")
    mod._hook = None
    def set_axon_ntff_profile_hook(h):
        mod._hook = h
    def get_axon_ntff_profile_hook():
        return mod._hook
    mod.set_axon_ntff_profile_hook = set_axon_ntff_profile_hook
    mod.get_axon_ntff_profile_hook = get_axon_ntff_profile_hook
    sys.modules["antenv.axon_hooks"] = mod
    antenv.axon_hooks = mod
    try:
        from trn_agent_boot.trn_boot import _ntff_profile_via_ctypes
        hook = _ntff_profile_via_ctypes("/opt/axon/libaxon_pjrt.so")
        if hook is not None:
            mod._hook = hook
    except Exception:
        pass


def _get_nc():
    global _cached_nc
    if _cached_nc is None:
        _cached_nc = _build_program()
    return _cached_nc


last_results = None  # BassKernelResults of the most recent run (for profiling)


def kernel(word, hidden, cell, encoder_output, max_source_len,
           embed_table, W_ih, W_hh, b_ih, b_hh,
           attn_W, attn_b, v, proj_W, proj_b, out_W, out_b,
           trace=False):
    global last_results
    word = np.asarray(word).astype(np.int32)
    hidden = np.asarray(hidden, dtype=np.float32)
    cell = np.asarray(cell, dtype=np.float32)
    enc = np.asarray(encoder_output, dtype=np.float32)
    embed_table = np.ascontiguousarray(np.asarray(embed_table, dtype=np.float32))
    W_ihT = np.ascontiguousarray(np.asarray(W_ih, dtype=np.float32).T)
    W_hhT = np.ascontiguousarray(np.asarray(W_hh, dtype=np.float32).T)
    bias_g = np.ascontiguousarray(
        (np.asarray(b_ih, dtype=np.float32)
         + np.asarray(b_hh, dtype=np.float32)).reshape(8, 128).T)
    attn_W = np.asarray(attn_W, dtype=np.float32)
    W_encT = np.ascontiguousarray(attn_W[:, :H2].T)
    W_hT = np.ascontiguousarray(attn_W[:, H2:].T)
    attn_b2 = np.ascontiguousarray(
        np.asarray(attn_b, dtype=np.float32).reshape(2, 128).T)
    v2 = np.ascontiguousarray(np.asarray(v, dtype=np.float32).reshape(2, 128).T)
    proj_WT = np.ascontiguousarray(np.asarray(proj_W, dtype=np.float32).T)
    proj_b1 = np.ascontiguousarray(
        np.asarray(proj_b, dtype=np.float32).reshape(1, H))
    out_W = np.asarray(out_W, dtype=np.float32)
    out_b = np.asarray(out_b, dtype=np.float32)

    in_maps = []
    for c in range(NCORES):
        bs = slice(c * BC, (c + 1) * BC)
        vs = slice(c * VC, (c + 1) * VC)

        in_maps.append({
            "word": np.ascontiguousarray(word[bs]),
            "embed": embed_table,
            "hiddenT": np.ascontiguousarray(hidden[bs].T),
            "cellT": np.ascontiguousarray(cell[bs].T),
            "encT": np.ascontiguousarray(enc[bs].reshape(RC, H2).T),
            "W_ihT": W_ihT,
            "W_hhT": W_hhT,
            "bias_g": bias_g,
            "W_encT": W_encT,
            "W_hT": W_hT,
            "attn_b2": attn_b2,
            "v2": v2,
            "proj_WT": proj_WT,
            "proj_b1": proj_b1,
            "out_WT": np.ascontiguousarray(out_W[vs].T),
            "out_b1v": np.ascontiguousarray(out_b[vs].reshape(1, VC)),
        })

    nc = _get_nc()
    if trace:
        _ensure_ntff_hook()
    res = run_bass_kernel_spmd(nc, in_maps, core_ids=list(range(NCORES)),
                               trace=trace)
    last_results = res

    word_dist = np.empty((B, V), dtype=np.float32)
    h_t = np.empty((B, H), dtype=np.float32)
    c_t = np.empty((B, H), dtype=np.float32)
    for c in range(NCORES):
        out = res.results[c]
        word_dist[:, c * VC:(c + 1) * VC] = out["word_dist"]
        h_t[c * BC:(c + 1) * BC] = out["h_t"]
        c_t[c * BC:(c + 1) * BC] = out["c_t"]
    return (word_dist, h_t, c_t)
